# revision 75
# baseline (speedup 1.0000x reference)
"""Trainium2 Bass kernel for NormalAttention (embedded gaussian, non-local block).

Reference computation per batch sample b (B=8, C=256, Ck=64, N=48*48=2304):
    q = Wq @ x + bq            (64, N)
    k = Wk @ x + bk            (64, N)
    e[i,j] = q[:,i] . k[:,j]   (N, N)
    E = exp(e);  E[i,j] /= sum_j E[i,j]
    v = Wv @ x + bv            (256, N)
    att[c,j] = sum_i v[c,i] * E[i,j]
    out = Wg @ att + bg        (256, N)

Sharding: pure data parallel, one batch sample per NeuronCore (8 cores).

Per-core kernel structure (fp8 attention rework; 82.0us -> 66.0us in the
TimelineSim cost model, rel err 1.28e-2 vs the 2e-2 gate):
  - Q/K/V projections and the energy matmul stay bf16 (error budget),
    but exp() writes E directly as fp8 e4m3 (max exp ~114 < 240).  The
    big V@E matmul then runs in fp8 DoubleRow perf mode: each matmul
    contracts TWO 128-row i-chunks at 0.5 PE-cycles/column, 4x the
    bf16 accumulate chain (34.6us -> 17.3us of PE).
  - fp8 quantization of V would pass ~1.5% error straight to the
    output, so V runs as two DoubleRow passes: vt8 = fp8(vt*invs) plus
    a residual vtr = fp8(vt*invs - vt8), accumulated into the same
    PSUM group.  Only E's quantization error remains (~1.3e-2 rel).
  - vt*invs is ~3e-4 scale (underflows fp8), so the combined V weight
    (Wg@Wv, gamma conv folded in on the host) is pre-scaled by 4096
    (power of two, exact in bf16); the final output op un-scales by
    1/4096 fused into the gamma-bias add.
  - pass 1 is ACT-paced at 2477ns/chunk (exp is irreducible: 2304
    elems/chunk at 0.833ns/elem + 2 PSUM->SBUF op bubbles + one accum
    read).  Energy PSUM: (128,1536)+(128,768) single-buffered (5
    banks); PE refills half A while ACT exps half B.  Row sums: half A
    rides the exp's ACT accumulator, half B is a DVE tensor_reduce
    (both halves ride ACT for the last two chunks so the DVE chain
    that gates every group's final pair isn't queued behind a reduce).
  - att groups: 2 stream their DoubleRow accumulation inline during
    pass 1 (st + sm banks).  The other 8 get "partial sessions": the
    V-projection bank is time-shared, and one pending group per
    session chunk accumulates up to PAIR_CAP of its available pairs
    there, then a DVE op dumps (partial*1/4096 + gbias) to SBUF.  The
    tail plays the remaining pairs out of the freed banks (big chains
    first on the earliest-freed banks) and combines with the dumped
    partial; outputs store as bf16 (halves the closing DMA stream) and
    are converted back to f32 on the host.
  - head: the two DMA queues are interleaved (a queue issues one
    dma_start per ~650ns and a DMA reaches the engine ~1.7us after
    issue, so queue depth directly prices the x pieces that gate the
    first Q/K tiles); PE warmup matmuls + a 1-element exp (prefetches
    the ACT function table) run under the DMAs.
"""

import os
import sys

sys.path.insert(0, "/opt/trn_rl_repo")

# NTFF tracing is unavailable through this container's axon client; make sure
# a stray BASS_TRACE in the environment can't break the execution path.
os.environ["BASS_NEVER_TRACE"] = "1"

# This kernel executes through the axon-proxied PJRT backend. If the caller's
# environment pinned jax to CPU (common for reference-side runs), drop the pin
# before jax initializes so the TRN2 devices stay discoverable.
_jp = os.environ.get("JAX_PLATFORMS")
if _jp and "axon" not in _jp and "jax" not in sys.modules:
    os.environ.pop("JAX_PLATFORMS", None)

import numpy as np
import ml_dtypes

import concourse.bass as bass
import concourse.mybir as mybir
import concourse.tile as tile
from concourse import bacc
from concourse.bass_utils import run_bass_kernel_spmd

B, C, CK, H, W = 8, 256, 64, 48, 48
N = H * W            # 2304
P = 128
NI = N // P          # 18 i-chunks
NPAIR = NI // 2      # 9 DoubleRow chunk pairs
NCORES = 8

BF16 = mybir.dt.bfloat16
FP8 = mybir.dt.float8e4
F32 = mybir.dt.float32
AF = mybir.ActivationFunctionType
ALU = mybir.AluOpType
AX = mybir.AxisListType
DR = mybir.MatmulPerfMode.DoubleRow

VSCALE = 4096.0      # host-folded V scale so vt*invs fits fp8

# energy ping-pong PSUM split: (128,1536)=3 banks + (128,768)=2 banks.
# A is the accum_out half (row sum free on ACT); B's row sum runs on DVE,
# so keep B small (fp8 input disables the DVE fast modes).
EA, EB = 1536, 768
E_SPLITS = [
    (0, EA, "engA", [(0, 512), (512, 512), (1024, 512)]),
    (EA, EB, "engB", [(0, 512), (512, 256)]),
]
# j-tiling of the att output; must nest inside the A/B split
J_TILES = [(0, 512), (512, 512), (1024, 512), (1536, 512), (2048, 256)]
# att groups streamed during pass 1 (both inside half A)
STREAM_GROUPS = [(0, 0), (1, 0)]
# the other groups (oc, j0, jw, tail psum tag).  Each gets a partial
# session (chunk, group) covering up to PAIR_CAP of its leading pairs in
# the time-shared V bank; tag order matters in the tail -- engA frees
# first (after exp(17) reads half A), engB next; sm/st after the stream
# close-outs.  256-wide ones last (smallest final DMA).
TAIL_GROUPS = [
    (1, 2048, 256, "engA"),   # gi=0: 2nd on engA
    (0, 2048, 256, "v"),      # gi=1: no session; 1st on the V bank (dead
    (1, 1024, 512, "v"),      #       after psv17), closes right after the
    (0, 512, 512, "engA"),    #       streams; gi=2 takes the bank second
    (1, 512, 512, "engB"),
    (0, 1024, 512, "st"),
    (0, 1536, 512, "sm"),
    (1, 1536, 512, "engB"),
]
# (chunk, tail-group index); pairs available at chunk k: 0..(k-2)//2.
# No sessions after chunk 14: they would sit in front of psv(16)/psv(17)
# in the V-bank WAR chain and push the pair-8 gating tiles out.
SESSIONS = [(6, 3), (8, 4), (10, 5), (11, 6), (12, 2), (13, 0), (14, 7)]
# groups whose partial is dumped RAW and preloaded into the tail PSUM by an
# (idle) ACT copy, so their final store is a plain ACT emit instead of a
# DVE combine -- these are the last closures, and DVE is the end-chain's
# bottleneck.  Only safe when the bank's previous group was full-width:
# a start=False chain after a 256-wide group would hit the stale half of
# the 2KB pending-zero region (g2 follows the 256-wide nsess -> excluded).
PRELOAD = set()
# tail emission order: each bank runs its BIG chain first (engA/engB free
# ~2.5us before sm/st, which wait on the stream close-outs), small chains
# second, so the last closure -- which pays the full emit + DMA init +
# transfer + completion latency (~4.5us) -- comes as early as possible
TAIL_ORDER = [3, 4, 6, 5, 7, 2, 0]
PAIR_CAP = 5

N_WARM = 7           # PE warmup matmuls issued under the input DMAs


def _build_nc():
    nc = bacc.Bacc("TRN2", target_bir_lowering=False, debug=False,
                   num_devices=NCORES)

    x_d = nc.dram_tensor("x", [2, P, N], BF16, kind="ExternalInput")
    wqk_d = nc.dram_tensor("wqk", [P, 2 * P], BF16, kind="ExternalInput")
    wrest_d = nc.dram_tensor("wrest", [P, 2 * C], BF16, kind="ExternalInput")
    fblob_d = nc.dram_tensor("fblob", [P, C + 4], F32, kind="ExternalInput")
    out_d = nc.dram_tensor("out", [2, P, N], BF16, kind="ExternalOutput")
    warm_d = nc.dram_tensor("warm", [P, 1], F32, kind="ExternalOutput")

    with tile.TileContext(nc) as tc:
        with (
            tc.tile_pool(name="consts", bufs=1) as consts,
            tc.tile_pool(name="big", bufs=1) as big,
            tc.tile_pool(name="work", bufs=6) as work,
            tc.tile_pool(name="ps_big", bufs=1, space="PSUM") as ps_big,
            tc.tile_pool(name="ps_v", bufs=1, space="PSUM") as ps_v,
            tc.tile_pool(name="ps_sm", bufs=1, space="PSUM") as ps_sm,
            tc.tile_pool(name="ps_st", bufs=1, space="PSUM") as ps_st,
        ):
            # ---------------- inputs ----------------
            # wqk + fblob first (small, gate the whole Q/K chain), then x in
            # three 768-column pieces -- energy(0) half A needs only the
            # first two.  Spread across queues so pieces land in parallel.
            xt = big.tile([P, 2, N], BF16)
            fblob = consts.tile([P, C + 4], F32)
            wqk = consts.tile([P, 2 * P], BF16)
            x_r = x_d[:].rearrange("c p n -> p c n")
            # interleave the two DMA queues: each queue issues one dma_start
            # every ~650ns and a DMA can't hit the engine until issue+~1.7us,
            # so queue depth directly prices the head-critical x pieces
            wrest = consts.tile([P, 2 * C], BF16)
            nc.scalar.dma_start(xt[:, :, 0:512], x_r[:, :, 0:512])
            nc.sync.dma_start(fblob, fblob_d[:])
            nc.scalar.dma_start(xt[:, :, 1024:1536], x_r[:, :, 1024:1536])
            nc.sync.dma_start(wqk[:], wqk_d[:])
            nc.sync.dma_start(xt[:, :, 512:1024], x_r[:, :, 512:1024])
            nc.sync.dma_start(xt[:, :, 1536:N], x_r[:, :, 1536:N])
            # wrest is only needed for the V projections (after the first
            # exps) -- it rides last so it never steals an engine slot
            nc.scalar.dma_start(wrest[:], wrest_d[:])

            # ---------------- PE warmup under the input DMAs ----------------
            dummy = consts.tile([P, 512], BF16)
            nc.vector.memset(dummy[:], 0)
            warm_sb = consts.tile([P, 1], F32)
            # 1-element exp: forces the implicit ACT_TABLE_LOAD (~1.3us) to
            # run at t~0 under the DMAs instead of gating the first q-bias
            nc.scalar.activation(warm_sb[0:1, 0:1], dummy[0:1, 0:1], AF.Exp)
            psd = ps_sm.tile([P, 512], F32, tag="sm")
            for w in range(N_WARM):
                nc.tensor.matmul(psd[:], dummy[:, :P], dummy[:],
                                 start=(w == 0), stop=(w == N_WARM - 1))
            nc.vector.tensor_copy(warm_sb, psd[:, 0:1])
            nc.sync.dma_start(warm_d[:], warm_sb)

            # combined projection weight W_comb = (Wg @ Wv)^T * 4096: the
            # gamma 1x1 conv is folded into the V projection on the host, so
            # pass-2 outputs are final after a 1/4096 un-scale + gamma bias
            def wv(c):
                return wrest[:, c * C:(c + 1) * C]

            qb = fblob[0:CK, 0:1]
            kb = fblob[0:CK, 1:2]
            vb = fblob[:, 4:C + 4]
            gbias = fblob[:, 2:4]

            # ---------------- Q / K projections ----------------
            q_t = big.tile([CK, N], BF16)
            k_t = big.tile([CK, N], BF16)

            # k-bias lands on DVE, q-bias on ACT (parallel PSUM->SBUF chains;
            # the energy matmuls are gated mostly on k_t). The q-bias for a
            # j-tile can be deferred (only q_t[:, k*128:(k+1)*128] gates
            # chunk k's energy row).
            def psk_mms(j0, jw, pool, tag, on_act=False):
                psk = pool.tile([P, 512], F32, tag=tag, name="psk")
                for c in range(2):
                    nc.tensor.matmul(psk[:CK, :jw],
                                     wqk[:, c * P + CK:(c + 1) * P],
                                     xt[:, c, j0:j0 + jw],
                                     start=(c == 0), stop=(c == 1))
                if on_act:
                    nc.scalar.activation(k_t[:, j0:j0 + jw], psk[:CK, :jw],
                                         AF.Identity, bias=kb)
                else:
                    nc.vector.tensor_scalar_add(k_t[:, j0:j0 + jw],
                                                psk[:CK, :jw], kb)

            def psq_mms(j0, jw, pool, tag, on_act=False):
                psq = pool.tile([P, 512], F32, tag=tag, name="psq")
                for c in range(2):
                    nc.tensor.matmul(psq[:CK, :jw], wqk[:, c * P:c * P + CK],
                                     xt[:, c, j0:j0 + jw],
                                     start=(c == 0), stop=(c == 1))
                if on_act:
                    # ACT is idle during the head; q_t[:, :128] gates exp(0)
                    nc.scalar.activation(q_t[:, j0:j0 + jw], psq[:CK, :jw],
                                         AF.Identity, bias=qb)
                else:
                    nc.vector.tensor_scalar_add(q_t[:, j0:j0 + jw],
                                                psq[:CK, :jw], qb)

            # shared big SBUF tensors
            vt = big.tile([P, NI, C], BF16)       # (Wg@Wv@x + b) * 4096
            vt8 = big.tile([P, NI, C], FP8)       # fp8(vt * invs)
            vtr = big.tile([P, NI, C], FP8)       # fp8(vt * invs - vt8)
            expA = big.tile([P, NI, EA], FP8)
            expB = big.tile([P, NI, EB], FP8)
            s_half = big.tile([P, NI, 2], F32)
            invs = big.tile([P, NI], F32)
            partials = big.tile([P, len(SESSIONS), 512], F32)

            eps_of = {}

            def emit_energy(kk, part):
                (base, width, tag, subs) = E_SPLITS[part]
                eps = ps_big.tile([P, width], F32, tag=tag, name=f"eps{part}")
                for (o0, ow) in subs:
                    nc.tensor.matmul(
                        eps[:, o0:o0 + ow],
                        q_t[:, kk * P:(kk + 1) * P],
                        k_t[:, base + o0:base + o0 + ow],
                        start=True, stop=True)
                eps_of.setdefault(kk, [None, None])[part] = eps

            # Q/K for j < 1536, then E(0) half A right away; rest of Q/K,
            # then E(0) half B -- gets the first exp started ASAP.  The
            # rotation cycles the v/sm/st banks (energy banks are busy from
            # E(0) on; everything else is head-dead by pass 1).
            psk_mms(*J_TILES[0], pool=ps_v, tag="v")
            psq_mms(*J_TILES[0], pool=ps_sm, tag="sm", on_act=True)
            psk_mms(*J_TILES[1], pool=ps_st, tag="st", on_act=True)
            psk_mms(*J_TILES[2], pool=ps_v, tag="v")
            emit_energy(0, 0)
            # k before q: the j3/j4 k-biases gate energy(0) half B, while the
            # q-biases for j-tiles >= 1 aren't needed until chunks 4+
            psk_mms(*J_TILES[3], pool=ps_sm, tag="sm")
            psk_mms(*J_TILES[4], pool=ps_v, tag="v")
            emit_energy(0, 1)
            psq_mms(*J_TILES[1], pool=ps_sm, tag="sm")
            psq_mms(*J_TILES[2], pool=ps_v, tag="v")
            psq_mms(*J_TILES[3], pool=ps_sm, tag="sm")
            psq_mms(*J_TILES[4], pool=ps_v, tag="v")

            def exp_slice(m, j0, jw):
                if j0 + jw <= EA:
                    return expA[:, 2 * m:2 * m + 2, j0:j0 + jw]
                return expB[:, 2 * m:2 * m + 2, j0 - EA:j0 - EA + jw]

            def pair_mms(dst_ap, oc, j0, jw, m, first, last):
                for pi, vsrc in enumerate((vt8, vtr)):
                    nc.tensor.matmul(
                        dst_ap,
                        vsrc[:, 2 * m:2 * m + 2, oc * P:(oc + 1) * P],
                        exp_slice(m, j0, jw),
                        perf_mode=DR,
                        start=(first and pi == 0),
                        stop=(last and pi == 1))

            # ---------------- pass 1 pipeline ----------------
            # per chunk k: exp both halves on ACT (fp8 out, half-A row sum in
            # the ACT accumulator), half-B row sum + invs + the fp8 V tiles
            # on DVE, next chunk's energy + V projection + streamed att on
            # PE.  At even chunks >= 4 one pending att group runs its partial
            # session in the V bank.
            st_tiles = []
            sess_done = {}        # tail-group index -> (#pairs played, si)
            def v_proj(k):
                # demoted early on so it only fills PE idle slots; prompt
                # near the end where its vt gates the pair-8 fp8 tiles
                with tc.high_priority(offset=(-100000 if k < 12 else 0)):
                    psv = ps_v.tile([P, 512], F32, tag="v", name="psv")
                    for c in range(2):
                        nc.tensor.matmul(psv[:, :C],
                                         xt[:, c, k * P:(k + 1) * P],
                                         wv(c), start=(c == 0), stop=(c == 1))
                    nc.vector.tensor_tensor(vt[:, k], psv[:, :C], vb, ALU.add)

            for k in range(NI):
                # V projection (emitted before vt8/vtr so the vt read-after-
                # write ordering is program order).  Chunk 17's is hoisted
                # next to 16's: its vt-add then precedes the chunk-16 fp8
                # tiles in the DVE queue, compressing the post-exp(17) chain
                # that gates every pair-8 matmul.
                if k < NI - 1:
                    v_proj(k)
                if k == NI - 2:
                    v_proj(NI - 1)
                last2 = (k >= NI - 2)
                nc.scalar.activation(
                    out=expA[:, k, :], in_=eps_of[k][0][:], func=AF.Exp,
                    accum_out=s_half[:, k, 0:1])
                # for the final two chunks the B row sum rides ACT too: the
                # DVE reduce would otherwise gate the pair-8 matmuls that
                # every att group needs before it can close
                nc.scalar.activation(
                    out=expB[:, k, :], in_=eps_of[k][1][:], func=AF.Exp,
                    accum_out=(s_half[:, k, 1:2] if last2 else None))
                if not last2:
                    nc.vector.tensor_reduce(
                        s_half[:, k, 1:2], expB[:, k, :], axis=AX.X,
                        op=ALU.add)
                nc.vector.tensor_tensor(invs[:, k:k + 1], s_half[:, k, 0:1],
                                        s_half[:, k, 1:2], ALU.add)
                nc.vector.reciprocal(invs[:, k:k + 1], invs[:, k:k + 1])
                # fp8 V tiles: vt8 = fp8(vt*invs), vtr = fp8(vt*invs - vt8)
                nc.vector.tensor_scalar(
                    vt8[:, k], vt[:, k], invs[:, k:k + 1], None, ALU.mult)
                nc.vector.scalar_tensor_tensor(
                    vtr[:, k], vt[:, k], invs[:, k:k + 1], vt8[:, k],
                    ALU.mult, ALU.subtract)
                # PE order: E(k+1)A first (unblocks the next exp ASAP)
                if k + 1 < NI:
                    emit_energy(k + 1, 0)
                if k >= 2 and k % 2 == 0:
                    m = (k - 2) // 2
                    if m == 0:
                        st_tiles = [
                            ps_st.tile([P, 512], F32, tag="st", name="st_g0"),
                            ps_sm.tile([P, 512], F32, tag="sm", name="st_g1"),
                        ]
                    for gi, (oc, j0) in enumerate(STREAM_GROUPS):
                        pair_mms(st_tiles[gi][:], oc, j0, 512, m,
                                 first=(m == 0), last=False)
                if k + 1 < NI:
                    emit_energy(k + 1, 1)
                # partial session: one pending att group plays all its
                # available pairs in the V bank, dump lands in SBUF
                for si, (sk, gi) in enumerate(SESSIONS):
                    if sk != k:
                        continue
                    (oc, j0, jw, _) = TAIL_GROUPS[gi]
                    npairs = min((k - 2) // 2 + 1, PAIR_CAP)
                    sess_done[gi] = (npairs, si)
                    with tc.high_priority(offset=-100000):
                        ps_part = ps_v.tile([P, 512], F32, tag="v",
                                            name="ps_part")
                        for m in range(npairs):
                            pair_mms(ps_part[:, :jw], oc, j0, jw, m,
                                     first=(m == 0), last=(m == npairs - 1))
                        if gi in PRELOAD:
                            # raw dump; re-materialized into the tail PSUM
                            nc.vector.tensor_copy(
                                partials[:, si, :jw], ps_part[:, :jw])
                        else:
                            # dump: (partial * 1/4096 + gamma_bias) -> SBUF
                            nc.vector.tensor_scalar(
                                partials[:, si, :jw], ps_part[:, :jw],
                                1.0 / VSCALE, gbias[:, oc:oc + 1],
                                ALU.mult, ALU.add)

            # ---------------- pass 2 (tail) ----------------
            dma_q = [0]

            def emit_out(oc, j0, jw, psum_ap, si=None):
                # bf16 store (converted back to f32 on the host): halves the
                # end-of-kernel DMA stream, which is the tail's bottleneck
                ot = work.tile([P, 512], BF16, tag="out")
                if si is None:
                    # (att * 1/4096) + gamma_bias -- on ACT, which is idle
                    # once the exps are done (DVE is the tail's bottleneck)
                    nc.scalar.activation(
                        ot[:, :jw], psum_ap, AF.Identity,
                        bias=gbias[:, oc:oc + 1], scale=1.0 / VSCALE)
                else:
                    # (att_rest * 1/4096) + dumped partial (bias included)
                    nc.vector.scalar_tensor_tensor(
                        ot[:, :jw], psum_ap, 1.0 / VSCALE,
                        partials[:, si, :jw], ALU.mult, ALU.add)
                q = nc.sync if dma_q[0] % 2 == 0 else nc.scalar
                dma_q[0] += 1
                q.dma_start(out_d[oc, :, j0:j0 + jw], ot[:, :jw])

            def tail_pool(tag):
                return {"engA": ps_big, "engB": ps_big,
                        "sm": ps_sm, "st": ps_st, "v": ps_v}[tag]

            # the no-session group opens first (its bank frees earliest and
            # pairs 0..7 don't need the chunk-16/17 fp8 tiles)...
            (noc, nj0, njw, ntag) = TAIL_GROUPS[1]
            nsess = tail_pool(ntag).tile([P, 512], F32, tag=ntag, name="aps")
            for m in range(NPAIR - 1):
                pair_mms(nsess[:, :njw], noc, nj0, njw, m,
                         first=(m == 0), last=False)

            # stream groups: final pair + store
            for gi, (oc, j0) in enumerate(STREAM_GROUPS):
                pair_mms(st_tiles[gi][:], oc, j0, 512, NPAIR - 1,
                         first=False, last=True)
                emit_out(oc, j0, 512, st_tiles[gi][:])

            # the no-session group closes right behind the streams so the V
            # bank can host one more chain
            pair_mms(nsess[:, :njw], noc, nj0, njw, NPAIR - 1,
                     first=False, last=True)
            emit_out(noc, nj0, njw, nsess[:, :njw])

            # remaining groups: play the pairs their session didn't cover
            for gi in TAIL_ORDER:
                (oc, j0, jw, tag) = TAIL_GROUPS[gi]
                aps = tail_pool(tag).tile([P, 512], F32, tag=tag, name="aps")
                m0, si = sess_done.get(gi, (0, None))
                pre = gi in PRELOAD and si is not None
                if pre:
                    # seed the accumulator with the session's raw partial
                    # (Identity: same ACT table family as Exp -- no reload)
                    nc.scalar.activation(aps[:, :jw], partials[:, si, :jw],
                                         AF.Identity)
                for m in range(m0, NPAIR):
                    pair_mms(aps[:, :jw], oc, j0, jw, m,
                             first=(m == m0 and not pre),
                             last=(m == NPAIR - 1))
                emit_out(oc, j0, jw, aps[:, :jw], si=(None if pre else si))

    nc.compile()
    return nc


_NC_CACHE = []


def _get_nc():
    if not _NC_CACHE:
        _NC_CACHE.append(_build_nc())
    return _NC_CACHE[0]


def _prep_inputs(x, query_weight, query_bias, key_weight, key_bias,
                 value_weight, value_bias, gamma_weight, gamma_bias):
    bf16 = ml_dtypes.bfloat16
    x = np.asarray(x, np.float32).reshape(B, C, N)
    qw = np.asarray(query_weight, np.float32)[:, :, 0, 0]   # (64, 256)
    kw = np.asarray(key_weight, np.float32)[:, :, 0, 0]     # (64, 256)
    vw = np.asarray(value_weight, np.float32)[:, :, 0, 0]   # (256, 256)
    gw = np.asarray(gamma_weight, np.float32)[:, :, 0, 0]   # (256, 256)

    # wqk[p, c*128+m] = W_cat^T[c*128+p, m]  (W_cat = [Wq; Wk], (128, 256))
    wcat_t = np.concatenate([qw, kw], axis=0).T              # (256, 128)
    wqk = np.ascontiguousarray(
        wcat_t.reshape(2, P, P).transpose(1, 0, 2).reshape(P, 2 * P))

    # the gamma 1x1 conv folds into the V projection:
    #   out = Wg @ (VS^T E) + bg = ((Wv^T Wg^T-projected X)^T-scaled E) + bg
    # so the device projects x with W_comb = (Wg @ Wv)^T * 4096 (the scale
    # keeps vt*invs inside fp8 range; un-scaled in the output bias op) and
    # the value bias becomes bvg = Wg @ bv * 4096.
    w_comb = (gw @ vw).T * VSCALE                           # (c_in, o)
    wrest = np.ascontiguousarray(
        w_comb.reshape(2, P, C).transpose(1, 0, 2).reshape(P, 2 * C))
    bvg = gw @ np.asarray(value_bias, np.float32) * VSCALE

    fblob = np.zeros((P, C + 4), np.float32)
    fblob[0:CK, 0] = np.asarray(query_bias, np.float32)
    fblob[0:CK, 1] = np.asarray(key_bias, np.float32)
    fblob[:, 2:4] = np.asarray(gamma_bias, np.float32).reshape(2, P).T
    fblob[:, 4:C + 4] = bvg[None, :]

    base = {
        "wqk": wqk.astype(bf16),
        "wrest": wrest.astype(bf16),
        "fblob": fblob,
    }
    in_maps = []
    for b in range(B):
        m = dict(base)
        m["x"] = x[b].reshape(2, P, N).astype(bf16)
        in_maps.append(m)
    return in_maps


def kernel(x, query_weight, query_bias, key_weight, key_bias,
           value_weight, value_bias, gamma_weight, gamma_bias, k):
    assert int(k) == C // CK
    in_maps = _prep_inputs(x, query_weight, query_bias, key_weight, key_bias,
                           value_weight, value_bias, gamma_weight, gamma_bias)
    nc = _get_nc()
    res = run_bass_kernel_spmd(nc, in_maps, core_ids=list(range(NCORES)))

    out = np.empty((B, C, H, W), np.float32)
    for b in range(B):
        out[b] = res.results[b]["out"].reshape(C, H, W)
    return out


# revision 76
# speedup vs baseline: 1.0318x; 1.0318x over previous
"""Trainium2 Bass kernel for NormalAttention (embedded gaussian, non-local block).

Reference computation per batch sample b (B=8, C=256, Ck=64, N=48*48=2304):
    q = Wq @ x + bq            (64, N)
    k = Wk @ x + bk            (64, N)
    e[i,j] = q[:,i] . k[:,j]   (N, N)
    E = exp(e);  E[i,j] /= sum_j E[i,j]
    v = Wv @ x + bv            (256, N)
    att[c,j] = sum_i v[c,i] * E[i,j]
    out = Wg @ att + bg        (256, N)

Sharding: pure data parallel, one batch sample per NeuronCore (8 cores).

Per-core kernel structure (fp8 attention rework; 82.0us -> 66.0us in the
TimelineSim cost model, rel err 1.28e-2 vs the 2e-2 gate):
  - Q/K/V projections and the energy matmul stay bf16 (error budget),
    but exp() writes E directly as fp8 e4m3 (max exp ~114 < 240).  The
    big V@E matmul then runs in fp8 DoubleRow perf mode: each matmul
    contracts TWO 128-row i-chunks at 0.5 PE-cycles/column, 4x the
    bf16 accumulate chain (34.6us -> 17.3us of PE).
  - fp8 quantization of V would pass ~1.5% error straight to the
    output, so V runs as two DoubleRow passes: vt8 = fp8(vt*invs) plus
    a residual vtr = fp8(vt*invs - vt8), accumulated into the same
    PSUM group.  Only E's quantization error remains (~1.3e-2 rel).
  - vt*invs is ~3e-4 scale (underflows fp8), so the combined V weight
    (Wg@Wv, gamma conv folded in on the host) is pre-scaled by 4096
    (power of two, exact in bf16); the final output op un-scales by
    1/4096 fused into the gamma-bias add.
  - pass 1 is ACT-paced at 2477ns/chunk (exp is irreducible: 2304
    elems/chunk at 0.833ns/elem + 2 PSUM->SBUF op bubbles + one accum
    read).  Energy PSUM: (128,1536)+(128,768) single-buffered (5
    banks); PE refills half A while ACT exps half B.  Row sums: half A
    rides the exp's ACT accumulator, half B is a DVE tensor_reduce
    (both halves ride ACT for the last two chunks so the DVE chain
    that gates every group's final pair isn't queued behind a reduce).
  - att groups: 2 stream their DoubleRow accumulation inline during
    pass 1 (st + sm banks).  The other 8 get "partial sessions": the
    V-projection bank is time-shared, and one pending group per
    session chunk accumulates up to PAIR_CAP of its available pairs
    there, then a DVE op dumps (partial*1/4096 + gbias) to SBUF.  The
    tail plays the remaining pairs out of the freed banks (big chains
    first on the earliest-freed banks) and combines with the dumped
    partial; outputs store as bf16 (halves the closing DMA stream) and
    are converted back to f32 on the host.
  - head: the two DMA queues are interleaved (a queue issues one
    dma_start per ~650ns and a DMA reaches the engine ~1.7us after
    issue, so queue depth directly prices the x pieces that gate the
    first Q/K tiles); PE warmup matmuls + a 1-element exp (prefetches
    the ACT function table) run under the DMAs.
"""

import os
import sys

sys.path.insert(0, "/opt/trn_rl_repo")

# NTFF tracing is unavailable through this container's axon client; make sure
# a stray BASS_TRACE in the environment can't break the execution path.
os.environ["BASS_NEVER_TRACE"] = "1"

# This kernel executes through the axon-proxied PJRT backend. If the caller's
# environment pinned jax to CPU (common for reference-side runs), drop the pin
# before jax initializes so the TRN2 devices stay discoverable.
_jp = os.environ.get("JAX_PLATFORMS")
if _jp and "axon" not in _jp and "jax" not in sys.modules:
    os.environ.pop("JAX_PLATFORMS", None)

import numpy as np
import ml_dtypes

import concourse.bass as bass
import concourse.mybir as mybir
import concourse.tile as tile
from concourse import bacc
from concourse.bass_utils import run_bass_kernel_spmd

B, C, CK, H, W = 8, 256, 64, 48, 48
N = H * W            # 2304
P = 128
NI = N // P          # 18 i-chunks
NPAIR = NI // 2      # 9 DoubleRow chunk pairs
NCORES = 8

BF16 = mybir.dt.bfloat16
FP8 = mybir.dt.float8e4
F32 = mybir.dt.float32
AF = mybir.ActivationFunctionType
ALU = mybir.AluOpType
AX = mybir.AxisListType
DR = mybir.MatmulPerfMode.DoubleRow

VSCALE = 4096.0      # host-folded V scale so vt*invs fits fp8

# energy ping-pong PSUM split: (128,1536)=3 banks + (128,768)=2 banks.
# A is the accum_out half (row sum free on ACT); B's row sum runs on DVE,
# so keep B small (fp8 input disables the DVE fast modes).
EA, EB = 1536, 768
E_SPLITS = [
    (0, EA, "engA", [(0, 512), (512, 512), (1024, 512)]),
    (EA, EB, "engB", [(0, 512), (512, 256)]),
]
# j-tiling of the att output; must nest inside the A/B split
J_TILES = [(0, 512), (512, 512), (1024, 512), (1536, 512), (2048, 256)]
# att groups streamed during pass 1 (both inside half A)
STREAM_GROUPS = [(0, 0), (1, 0)]
# the other groups (oc, j0, jw, tail psum tag).  Each gets a partial
# session (chunk, group) covering up to PAIR_CAP of its leading pairs in
# the time-shared V bank; tag order matters in the tail -- engA frees
# first (after exp(17) reads half A), engB next; sm/st after the stream
# close-outs.  256-wide ones last (smallest final DMA).
TAIL_GROUPS = [
    (1, 2048, 256, "engA"),   # gi=0: 2nd on engA
    (0, 2048, 256, "v"),      # gi=1: no session; 1st on the V bank (dead
    (1, 1024, 512, "v"),      #       after psv17), closes right after the
    (0, 512, 512, "engA"),    #       streams; gi=2 takes the bank second
    (1, 512, 512, "engB"),
    (0, 1024, 512, "st"),
    (0, 1536, 512, "sm"),
    (1, 1536, 512, "engB"),
]
# (chunk, tail-group index); pairs available at chunk k: 0..(k-2)//2.
# No sessions after chunk 14: they would sit in front of psv(16)/psv(17)
# in the V-bank WAR chain and push the pair-8 gating tiles out.
SESSIONS = [(4, 3), (6, 4), (8, 5), (10, 6), (12, 2), (13, 0), (14, 7)]
# groups whose partial is dumped RAW and preloaded into the tail PSUM by an
# (idle) ACT copy, so their final store is a plain ACT emit instead of a
# DVE combine -- these are the last closures, and DVE is the end-chain's
# bottleneck.  Only safe when the bank's previous group was full-width:
# a start=False chain after a 256-wide group would hit the stale half of
# the 2KB pending-zero region (g2 follows the 256-wide nsess -> excluded).
PRELOAD = set()
# tail emission order: each bank runs its BIG chain first (engA/engB free
# ~2.5us before sm/st, which wait on the stream close-outs), small chains
# second, so the last closure -- which pays the full emit + DMA init +
# transfer + completion latency (~4.5us) -- comes as early as possible
TAIL_ORDER = [3, 4, 6, 5, 7, 2, 0]
PAIR_CAP = 5

N_WARM = 7           # PE warmup matmuls issued under the input DMAs


def _build_nc():
    nc = bacc.Bacc("TRN2", target_bir_lowering=False, debug=False,
                   num_devices=NCORES)

    x_d = nc.dram_tensor("x", [2, P, N], BF16, kind="ExternalInput")
    wqk_d = nc.dram_tensor("wqk", [P, 2 * P], BF16, kind="ExternalInput")
    wrest_d = nc.dram_tensor("wrest", [P, 2 * C], BF16, kind="ExternalInput")
    fblob_d = nc.dram_tensor("fblob", [P, C + 4], F32, kind="ExternalInput")
    out_d = nc.dram_tensor("out", [2, P, N], BF16, kind="ExternalOutput")
    warm_d = nc.dram_tensor("warm", [P, 1], F32, kind="ExternalOutput")

    with tile.TileContext(nc) as tc:
        with (
            tc.tile_pool(name="consts", bufs=1) as consts,
            tc.tile_pool(name="big", bufs=1) as big,
            tc.tile_pool(name="work", bufs=6) as work,
            tc.tile_pool(name="ps_big", bufs=1, space="PSUM") as ps_big,
            tc.tile_pool(name="ps_v", bufs=1, space="PSUM") as ps_v,
            tc.tile_pool(name="ps_sm", bufs=1, space="PSUM") as ps_sm,
            tc.tile_pool(name="ps_st", bufs=1, space="PSUM") as ps_st,
        ):
            # ---------------- inputs ----------------
            # wqk + fblob first (small, gate the whole Q/K chain), then x in
            # three 768-column pieces -- energy(0) half A needs only the
            # first two.  Spread across queues so pieces land in parallel.
            xt = big.tile([P, 2, N], BF16)
            fblob = consts.tile([P, C + 4], F32)
            wqk = consts.tile([P, 2 * P], BF16)
            x_r = x_d[:].rearrange("c p n -> p c n")
            # interleave the two DMA queues: each queue issues one dma_start
            # every ~650ns and a DMA can't hit the engine until issue+~1.7us,
            # so queue depth directly prices the head-critical x pieces
            wrest = consts.tile([P, 2 * C], BF16)
            nc.scalar.dma_start(xt[:, :, 0:512], x_r[:, :, 0:512])
            nc.sync.dma_start(fblob, fblob_d[:])
            nc.scalar.dma_start(xt[:, :, 1024:1536], x_r[:, :, 1024:1536])
            nc.sync.dma_start(wqk[:], wqk_d[:])
            nc.sync.dma_start(xt[:, :, 512:1024], x_r[:, :, 512:1024])
            nc.sync.dma_start(xt[:, :, 1536:N], x_r[:, :, 1536:N])
            # wrest is only needed for the V projections (after the first
            # exps) -- it rides last so it never steals an engine slot
            nc.scalar.dma_start(wrest[:], wrest_d[:])

            # ---------------- PE warmup under the input DMAs ----------------
            dummy = consts.tile([P, 512], BF16)
            nc.vector.memset(dummy[:], 0)
            warm_sb = consts.tile([P, 1], F32)
            # 1-element exp: forces the implicit ACT_TABLE_LOAD (~1.3us) to
            # run at t~0 under the DMAs instead of gating the first q-bias
            nc.scalar.activation(warm_sb[0:1, 0:1], dummy[0:1, 0:1], AF.Exp)
            psd = ps_sm.tile([P, 512], F32, tag="sm")
            for w in range(N_WARM):
                nc.tensor.matmul(psd[:], dummy[:, :P], dummy[:],
                                 start=(w == 0), stop=(w == N_WARM - 1))
            nc.vector.tensor_copy(warm_sb, psd[:, 0:1])
            nc.sync.dma_start(warm_d[:], warm_sb)

            # combined projection weight W_comb = (Wg @ Wv)^T * 4096: the
            # gamma 1x1 conv is folded into the V projection on the host, so
            # pass-2 outputs are final after a 1/4096 un-scale + gamma bias
            def wv(c):
                return wrest[:, c * C:(c + 1) * C]

            qb = fblob[0:CK, 0:1]
            kb = fblob[0:CK, 1:2]
            vb = fblob[:, 4:C + 4]
            gbias = fblob[:, 2:4]

            # ---------------- Q / K projections ----------------
            q_t = big.tile([CK, N], BF16)
            k_t = big.tile([CK, N], BF16)

            # k-bias lands on DVE, q-bias on ACT (parallel PSUM->SBUF chains;
            # the energy matmuls are gated mostly on k_t). The q-bias for a
            # j-tile can be deferred (only q_t[:, k*128:(k+1)*128] gates
            # chunk k's energy row).
            def psk_mms(j0, jw, pool, tag, on_act=False):
                psk = pool.tile([P, 512], F32, tag=tag, name="psk")
                for c in range(2):
                    nc.tensor.matmul(psk[:CK, :jw],
                                     wqk[:, c * P + CK:(c + 1) * P],
                                     xt[:, c, j0:j0 + jw],
                                     start=(c == 0), stop=(c == 1))
                if on_act:
                    nc.scalar.activation(k_t[:, j0:j0 + jw], psk[:CK, :jw],
                                         AF.Identity, bias=kb)
                else:
                    nc.vector.tensor_scalar_add(k_t[:, j0:j0 + jw],
                                                psk[:CK, :jw], kb)

            def psq_mms(j0, jw, pool, tag, on_act=False):
                psq = pool.tile([P, 512], F32, tag=tag, name="psq")
                for c in range(2):
                    nc.tensor.matmul(psq[:CK, :jw], wqk[:, c * P:c * P + CK],
                                     xt[:, c, j0:j0 + jw],
                                     start=(c == 0), stop=(c == 1))
                if on_act:
                    # ACT is idle during the head; q_t[:, :128] gates exp(0)
                    nc.scalar.activation(q_t[:, j0:j0 + jw], psq[:CK, :jw],
                                         AF.Identity, bias=qb)
                else:
                    nc.vector.tensor_scalar_add(q_t[:, j0:j0 + jw],
                                                psq[:CK, :jw], qb)

            # shared big SBUF tensors
            vt = big.tile([P, NI, C], BF16)       # (Wg@Wv@x + b) * 4096
            vt8 = big.tile([P, NI, C], FP8)       # fp8(vt * invs)
            vtr = big.tile([P, NI, C], FP8)       # fp8(vt * invs - vt8)
            expA = big.tile([P, NI, EA], FP8)
            expB = big.tile([P, NI, EB], FP8)
            s_half = big.tile([P, NI, 2], F32)
            invs = big.tile([P, NI], F32)
            partials = big.tile([P, len(SESSIONS), 512], F32)

            eps_of = {}

            def emit_energy(kk, part):
                (base, width, tag, subs) = E_SPLITS[part]
                eps = ps_big.tile([P, width], F32, tag=tag, name=f"eps{part}")
                for (o0, ow) in subs:
                    nc.tensor.matmul(
                        eps[:, o0:o0 + ow],
                        q_t[:, kk * P:(kk + 1) * P],
                        k_t[:, base + o0:base + o0 + ow],
                        start=True, stop=True)
                eps_of.setdefault(kk, [None, None])[part] = eps

            # Q/K for j < 1536, then E(0) half A right away; rest of Q/K,
            # then E(0) half B -- gets the first exp started ASAP.  The
            # rotation cycles the v/sm/st banks (energy banks are busy from
            # E(0) on; everything else is head-dead by pass 1).
            psk_mms(*J_TILES[0], pool=ps_v, tag="v")
            psq_mms(*J_TILES[0], pool=ps_sm, tag="sm", on_act=True)
            psk_mms(*J_TILES[1], pool=ps_st, tag="st", on_act=True)
            psk_mms(*J_TILES[2], pool=ps_v, tag="v")
            emit_energy(0, 0)
            # k before q: the j3/j4 k-biases gate energy(0) half B, while the
            # q-biases for j-tiles >= 1 aren't needed until chunks 4+
            psk_mms(*J_TILES[3], pool=ps_sm, tag="sm")
            psk_mms(*J_TILES[4], pool=ps_v, tag="v")
            emit_energy(0, 1)
            psq_mms(*J_TILES[1], pool=ps_sm, tag="sm")
            psq_mms(*J_TILES[2], pool=ps_v, tag="v")
            psq_mms(*J_TILES[3], pool=ps_sm, tag="sm")
            psq_mms(*J_TILES[4], pool=ps_v, tag="v")

            def exp_slice(m, j0, jw):
                if j0 + jw <= EA:
                    return expA[:, 2 * m:2 * m + 2, j0:j0 + jw]
                return expB[:, 2 * m:2 * m + 2, j0 - EA:j0 - EA + jw]

            def pair_mms(dst_ap, oc, j0, jw, m, first, last):
                for pi, vsrc in enumerate((vt8, vtr)):
                    nc.tensor.matmul(
                        dst_ap,
                        vsrc[:, 2 * m:2 * m + 2, oc * P:(oc + 1) * P],
                        exp_slice(m, j0, jw),
                        perf_mode=DR,
                        start=(first and pi == 0),
                        stop=(last and pi == 1))

            # ---------------- pass 1 pipeline ----------------
            # per chunk k: exp both halves on ACT (fp8 out, half-A row sum in
            # the ACT accumulator), half-B row sum + invs + the fp8 V tiles
            # on DVE, next chunk's energy + V projection + streamed att on
            # PE.  At even chunks >= 4 one pending att group runs its partial
            # session in the V bank.
            st_tiles = []
            sess_done = {}        # tail-group index -> (#pairs played, si)
            def v_proj(k):
                # demoted early on so it only fills PE idle slots; prompt
                # near the end where its vt gates the pair-8 fp8 tiles
                with tc.high_priority(offset=(-100000 if k < 12 else 0)):
                    psv = ps_v.tile([P, 512], F32, tag="v", name="psv")
                    for c in range(2):
                        nc.tensor.matmul(psv[:, :C],
                                         xt[:, c, k * P:(k + 1) * P],
                                         wv(c), start=(c == 0), stop=(c == 1))
                    nc.vector.tensor_tensor(vt[:, k], psv[:, :C], vb, ALU.add)

            for k in range(NI):
                # V projection (emitted before vt8/vtr so the vt read-after-
                # write ordering is program order).  Chunk 17's is hoisted
                # next to 16's: its vt-add then precedes the chunk-16 fp8
                # tiles in the DVE queue, compressing the post-exp(17) chain
                # that gates every pair-8 matmul.
                if k < NI - 1:
                    v_proj(k)
                if k == NI - 2:
                    v_proj(NI - 1)
                last2 = (k >= NI - 2)
                nc.scalar.activation(
                    out=expA[:, k, :], in_=eps_of[k][0][:], func=AF.Exp,
                    accum_out=s_half[:, k, 0:1])
                # for the final two chunks the B row sum rides ACT too: the
                # DVE reduce would otherwise gate the pair-8 matmuls that
                # every att group needs before it can close
                nc.scalar.activation(
                    out=expB[:, k, :], in_=eps_of[k][1][:], func=AF.Exp,
                    accum_out=(s_half[:, k, 1:2] if last2 else None))
                if not last2:
                    nc.vector.tensor_reduce(
                        s_half[:, k, 1:2], expB[:, k, :], axis=AX.X,
                        op=ALU.add)
                nc.vector.tensor_tensor(invs[:, k:k + 1], s_half[:, k, 0:1],
                                        s_half[:, k, 1:2], ALU.add)
                nc.vector.reciprocal(invs[:, k:k + 1], invs[:, k:k + 1])
                # fp8 V tiles: vt8 = fp8(vt*invs), vtr = fp8(vt*invs - vt8)
                nc.vector.tensor_scalar(
                    vt8[:, k], vt[:, k], invs[:, k:k + 1], None, ALU.mult)
                nc.vector.scalar_tensor_tensor(
                    vtr[:, k], vt[:, k], invs[:, k:k + 1], vt8[:, k],
                    ALU.mult, ALU.subtract)
                # PE order: E(k+1)A first (unblocks the next exp ASAP)
                if k + 1 < NI:
                    emit_energy(k + 1, 0)
                if k >= 2 and k % 2 == 0:
                    m = (k - 2) // 2
                    if m == 0:
                        st_tiles = [
                            ps_st.tile([P, 512], F32, tag="st", name="st_g0"),
                            ps_sm.tile([P, 512], F32, tag="sm", name="st_g1"),
                        ]
                    for gi, (oc, j0) in enumerate(STREAM_GROUPS):
                        pair_mms(st_tiles[gi][:], oc, j0, 512, m,
                                 first=(m == 0), last=False)
                if k + 1 < NI:
                    emit_energy(k + 1, 1)
                # partial session: one pending att group plays all its
                # available pairs in the V bank, dump lands in SBUF
                for si, (sk, gi) in enumerate(SESSIONS):
                    if sk != k:
                        continue
                    (oc, j0, jw, _) = TAIL_GROUPS[gi]
                    npairs = min((k - 2) // 2 + 1, PAIR_CAP)
                    sess_done[gi] = (npairs, si)
                    with tc.high_priority(offset=-100000):
                        ps_part = ps_v.tile([P, 512], F32, tag="v",
                                            name="ps_part")
                        for m in range(npairs):
                            pair_mms(ps_part[:, :jw], oc, j0, jw, m,
                                     first=(m == 0), last=(m == npairs - 1))
                        if gi in PRELOAD:
                            # raw dump; re-materialized into the tail PSUM
                            nc.vector.tensor_copy(
                                partials[:, si, :jw], ps_part[:, :jw])
                        else:
                            # dump: (partial * 1/4096 + gamma_bias) -> SBUF
                            nc.vector.tensor_scalar(
                                partials[:, si, :jw], ps_part[:, :jw],
                                1.0 / VSCALE, gbias[:, oc:oc + 1],
                                ALU.mult, ALU.add)

            # ---------------- pass 2 (tail) ----------------
            dma_q = [0]

            def emit_out(oc, j0, jw, psum_ap, si=None):
                # bf16 store (converted back to f32 on the host): halves the
                # end-of-kernel DMA stream, which is the tail's bottleneck
                ot = work.tile([P, 512], BF16, tag="out")
                if si is None:
                    # (att * 1/4096) + gamma_bias -- on ACT, which is idle
                    # once the exps are done (DVE is the tail's bottleneck)
                    nc.scalar.activation(
                        ot[:, :jw], psum_ap, AF.Identity,
                        bias=gbias[:, oc:oc + 1], scale=1.0 / VSCALE)
                else:
                    # (att_rest * 1/4096) + dumped partial (bias included)
                    nc.vector.scalar_tensor_tensor(
                        ot[:, :jw], psum_ap, 1.0 / VSCALE,
                        partials[:, si, :jw], ALU.mult, ALU.add)
                q = nc.sync if dma_q[0] % 2 == 0 else nc.scalar
                dma_q[0] += 1
                q.dma_start(out_d[oc, :, j0:j0 + jw], ot[:, :jw])

            def tail_pool(tag):
                return {"engA": ps_big, "engB": ps_big,
                        "sm": ps_sm, "st": ps_st, "v": ps_v}[tag]

            # the no-session group opens first (its bank frees earliest and
            # pairs 0..7 don't need the chunk-16/17 fp8 tiles)...
            (noc, nj0, njw, ntag) = TAIL_GROUPS[1]
            nsess = tail_pool(ntag).tile([P, 512], F32, tag=ntag, name="aps")
            for m in range(NPAIR - 1):
                pair_mms(nsess[:, :njw], noc, nj0, njw, m,
                         first=(m == 0), last=False)

            # stream groups: final pair + store
            for gi, (oc, j0) in enumerate(STREAM_GROUPS):
                pair_mms(st_tiles[gi][:], oc, j0, 512, NPAIR - 1,
                         first=False, last=True)
                emit_out(oc, j0, 512, st_tiles[gi][:])

            # the no-session group closes right behind the streams so the V
            # bank can host one more chain
            pair_mms(nsess[:, :njw], noc, nj0, njw, NPAIR - 1,
                     first=False, last=True)
            emit_out(noc, nj0, njw, nsess[:, :njw])

            # remaining groups: play the pairs their session didn't cover
            for gi in TAIL_ORDER:
                (oc, j0, jw, tag) = TAIL_GROUPS[gi]
                aps = tail_pool(tag).tile([P, 512], F32, tag=tag, name="aps")
                m0, si = sess_done.get(gi, (0, None))
                pre = gi in PRELOAD and si is not None
                if pre:
                    # seed the accumulator with the session's raw partial
                    # (Identity: same ACT table family as Exp -- no reload)
                    nc.scalar.activation(aps[:, :jw], partials[:, si, :jw],
                                         AF.Identity)
                for m in range(m0, NPAIR):
                    pair_mms(aps[:, :jw], oc, j0, jw, m,
                             first=(m == m0 and not pre),
                             last=(m == NPAIR - 1))
                emit_out(oc, j0, jw, aps[:, :jw], si=(None if pre else si))

    nc.compile()
    return nc


_NC_CACHE = []


def _get_nc():
    if not _NC_CACHE:
        _NC_CACHE.append(_build_nc())
    return _NC_CACHE[0]


def _prep_inputs(x, query_weight, query_bias, key_weight, key_bias,
                 value_weight, value_bias, gamma_weight, gamma_bias):
    bf16 = ml_dtypes.bfloat16
    x = np.asarray(x, np.float32).reshape(B, C, N)
    qw = np.asarray(query_weight, np.float32)[:, :, 0, 0]   # (64, 256)
    kw = np.asarray(key_weight, np.float32)[:, :, 0, 0]     # (64, 256)
    vw = np.asarray(value_weight, np.float32)[:, :, 0, 0]   # (256, 256)
    gw = np.asarray(gamma_weight, np.float32)[:, :, 0, 0]   # (256, 256)

    # wqk[p, c*128+m] = W_cat^T[c*128+p, m]  (W_cat = [Wq; Wk], (128, 256))
    wcat_t = np.concatenate([qw, kw], axis=0).T              # (256, 128)
    wqk = np.ascontiguousarray(
        wcat_t.reshape(2, P, P).transpose(1, 0, 2).reshape(P, 2 * P))

    # the gamma 1x1 conv folds into the V projection:
    #   out = Wg @ (VS^T E) + bg = ((Wv^T Wg^T-projected X)^T-scaled E) + bg
    # so the device projects x with W_comb = (Wg @ Wv)^T * 4096 (the scale
    # keeps vt*invs inside fp8 range; un-scaled in the output bias op) and
    # the value bias becomes bvg = Wg @ bv * 4096.
    w_comb = (gw @ vw).T * VSCALE                           # (c_in, o)
    wrest = np.ascontiguousarray(
        w_comb.reshape(2, P, C).transpose(1, 0, 2).reshape(P, 2 * C))
    bvg = gw @ np.asarray(value_bias, np.float32) * VSCALE

    fblob = np.zeros((P, C + 4), np.float32)
    fblob[0:CK, 0] = np.asarray(query_bias, np.float32)
    fblob[0:CK, 1] = np.asarray(key_bias, np.float32)
    fblob[:, 2:4] = np.asarray(gamma_bias, np.float32).reshape(2, P).T
    fblob[:, 4:C + 4] = bvg[None, :]

    base = {
        "wqk": wqk.astype(bf16),
        "wrest": wrest.astype(bf16),
        "fblob": fblob,
    }
    in_maps = []
    for b in range(B):
        m = dict(base)
        m["x"] = x[b].reshape(2, P, N).astype(bf16)
        in_maps.append(m)
    return in_maps


def kernel(x, query_weight, query_bias, key_weight, key_bias,
           value_weight, value_bias, gamma_weight, gamma_bias, k):
    assert int(k) == C // CK
    in_maps = _prep_inputs(x, query_weight, query_bias, key_weight, key_bias,
                           value_weight, value_bias, gamma_weight, gamma_bias)
    nc = _get_nc()
    res = run_bass_kernel_spmd(nc, in_maps, core_ids=list(range(NCORES)))

    out = np.empty((B, C, H, W), np.float32)
    for b in range(B):
        out[b] = res.results[b]["out"].reshape(C, H, W)
    return out
